# revision 1
# baseline (speedup 1.0000x reference)
"""Neighbourhood attention block (7x7 window) on 8 Trainium2 NeuronCores.

Full inputs -> full output. Sharding: core = b*4 + g owns batch b and query
rows 16g..16g+15 (all 6 heads). Each core gets a 24-row halo slice of x,
transposed to feature-major and laid out in column-major token order
(token = wc*24 + r, wc = padded column 0..71, r = local row 0..23) so that
every 16-col x 24-row key window is a contiguous 384-token run: key chunks
(128 keys) and query blocks (128 queries = 8 cols x 16 rows) are then plain
2D slices, as required for matmul stationary operands.

Softmax runs without max-subtraction (scores are O(1)): keys on partitions,
probs = exp(s/8) * mask01 (bf16); the denominator comes free from a
ones-column appended to V (PV output col 64 of each 65-col head slot);
reciprocal + normalization are per-partition ops on the token-major PV
output; attn is then PE-transposed to feature-major for the out-projection.
"""
import sys

sys.path.insert(0, "/opt/trn_rl_repo")

import numpy as np
import ml_dtypes

import concourse.bass as bass
import concourse.mybir as mybir
from concourse import bacc
from concourse.tile import TileContext
from concourse.bass_utils import run_bass_kernel_spmd
from concourse.bass import broadcast_tensor_aps

F32 = mybir.dt.float32
BF16 = mybir.dt.bfloat16
AF = mybir.ActivationFunctionType

D = 384
NH = 6
E = 64
NCORES = 8
TOK = 1728          # 72 padded cols x 24 rows, column-major
SCALE = 0.125       # 1/sqrt(64)


def emit(nc):
    xT = nc.dram_tensor("xT", [D, TOK], F32, kind="ExternalInput").ap()
    wqkvT = nc.dram_tensor("wqkvT", [D, 3 * D], F32, kind="ExternalInput").ap()
    woutT = nc.dram_tensor("woutT", [D, D], F32, kind="ExternalInput").ap()
    ident = nc.dram_tensor("ident", [128, 128], F32, kind="ExternalInput").ap()
    mask = nc.dram_tensor("mask", [128, 8 * 3 * 128], BF16, kind="ExternalInput").ap()
    out = nc.dram_tensor("out", [8, 128, D], F32, kind="ExternalOutput").ap()

    with TileContext(nc) as tc:
        with tc.tile_pool(name="persist", bufs=1) as pp:
            xT_sb = [pp.tile([128, TOK], F32, tag=f"xT{i}", name=f"xT{i}")
                     for i in range(3)]
            w1_sb = [pp.tile([128, 3 * D], F32, tag=f"w1{i}", name=f"w1{i}")
                     for i in range(3)]
            w2_sb = [pp.tile([128, D], F32, tag=f"w2{i}", name=f"w2{i}")
                     for i in range(3)]
            id_sb = pp.tile([128, 128], F32, tag="id", name="idsb")
            mk_sb = pp.tile([128, 8 * 3 * 128], BF16, tag="mk", name="mksb")
            qT_sb = pp.tile([128, 3 * 1024], F32, tag="qT", name="qTsb")
            kT_sb = [pp.tile([128, TOK], F32, tag=f"kT{i}", name=f"kT{i}")
                     for i in range(3)]
            v_sb = pp.tile([128, 24 * 390], BF16, tag="v", name="vsb")

            for i in range(3):
                nc.sync.dma_start(out=xT_sb[i][:], in_=xT[i * 128:(i + 1) * 128, :])
                nc.sync.dma_start(out=w1_sb[i][:], in_=wqkvT[i * 128:(i + 1) * 128, :])
                nc.sync.dma_start(out=w2_sb[i][:], in_=woutT[i * 128:(i + 1) * 128, :])
            nc.sync.dma_start(out=id_sb[:], in_=ident[:])
            nc.sync.dma_start(out=mk_sb[:], in_=mask[:])

            # ones-columns of v (col 64 of each 65-col head slot)
            vv = v_sb[:].rearrange("p (c h e) -> p c h e", h=NH, e=65)
            nc.gpsimd.memset(vv[:, :, :, 64:65], 1.0)

            # ---- qkv projections ----
            with tc.tile_pool(name="qkps", bufs=3, space="PSUM") as qkp:
                # q^T: owned tokens (cols 4..67, rows 3..18), col-major
                for f in range(3):
                    for t in range(2):
                        ps = qkp.tile([128, 512], F32, tag="qk", name="qkps")
                        for d in range(3):
                            xv = xT_sb[d][:].rearrange("p (w r) -> p w r", r=24)
                            nc.tensor.matmul(
                                ps[:],
                                lhsT=w1_sb[d][:, f * 128:(f + 1) * 128],
                                rhs=xv[:, 4 + 32 * t:4 + 32 * (t + 1), 3:19],
                                start=(d == 0), stop=(d == 2),
                            )
                        nc.vector.tensor_copy(
                            out=qT_sb[:, f * 1024 + t * 512:f * 1024 + (t + 1) * 512],
                            in_=ps[:])
                # k^T over all 1728 tokens (zero pads give k=0)
                for f in range(3):
                    for t in range(4):
                        w = 512 if t < 3 else 192
                        ps = qkp.tile([128, 512], F32, tag="qk", name="qkps")
                        for d in range(3):
                            nc.tensor.matmul(
                                ps[:, :w],
                                lhsT=w1_sb[d][:, 384 + f * 128:384 + (f + 1) * 128],
                                rhs=xT_sb[d][:, t * 512:t * 512 + w],
                                start=(d == 0), stop=(d == 2),
                            )
                        nc.vector.tensor_copy(
                            out=kT_sb[f][:, t * 512:t * 512 + w], in_=ps[:, :w])
                # v in key-chunk layout [128 keys, 6*65] per (bw, c)
                for ch in range(24):
                    bw, c = divmod(ch, 3)
                    k0 = 192 * bw + 128 * c
                    ps = qkp.tile([128, 384], F32, tag="vps", name="vps")
                    for d in range(3):
                        nc.tensor.matmul(
                            ps[:],
                            lhsT=xT_sb[d][:, k0:k0 + 128],
                            rhs=w1_sb[d][:, 768:1152],
                            start=(d == 0), stop=(d == 2),
                        )
                    nc.vector.tensor_copy(
                        out=vv[:, ch, :, 0:64],
                        in_=ps[:].rearrange("p (h e) -> p h e", e=64))

            # ---- attention + output projection ----
            with tc.tile_pool(name="spool", bufs=2, space="PSUM") as spool, \
                 tc.tile_pool(name="pvpool", bufs=1, space="PSUM") as pvpool, \
                 tc.tile_pool(name="trpool", bufs=1, space="PSUM") as trpool, \
                 tc.tile_pool(name="fpool", bufs=1, space="PSUM") as fpool, \
                 tc.tile_pool(name="work", bufs=6) as wp, \
                 tc.tile_pool(name="work2", bufs=2) as wp2:
                for bw in range(8):
                    e_tiles = []
                    for c in range(3):
                        e_sb = wp.tile([128, 768], BF16, tag="e", name="esb")
                        k0 = 192 * bw + 128 * c
                        m1 = mk_sb[:, (bw * 3 + c) * 128:(bw * 3 + c + 1) * 128]
                        m3 = m1.rearrange("p (o q) -> p o q", o=1)
                        for ph in range(3):
                            # pair tile: head-even -> bank 0 (cols 0:128),
                            # head-odd -> bank 1 (cols 512:640); one matmul
                            # group per bank (HW requirement)
                            sps = spool.tile([128, 1024], F32, tag="s",
                                             name="sps")
                            for par in range(2):
                                h = 2 * ph + par
                                nc.tensor.matmul(
                                    sps[:, par * 512:par * 512 + 128],
                                    lhsT=kT_sb[ph][par * 64:par * 64 + 64,
                                                   k0:k0 + 128],
                                    rhs=qT_sb[par * 64:par * 64 + 64,
                                              ph * 1024 + bw * 128:
                                              ph * 1024 + (bw + 1) * 128],
                                    start=True, stop=True,
                                    tile_position=(par * 64, 0),
                                )
                            sps3 = sps[:].rearrange(
                                "p (b q) -> p b q", q=512)[:, :, 0:128]
                            e3 = e_sb[:, ph * 256:(ph + 1) * 256].rearrange(
                                "p (b q) -> p b q", q=128)
                            nc.scalar.activation(out=e3, in_=sps3, func=AF.Exp,
                                                 scale=SCALE)
                            a, b = broadcast_tensor_aps(e3, m3)
                            nc.vector.tensor_mul(out=e3, in0=a, in1=b)
                        e_tiles.append(e_sb)
                    at = wp.tile([128, 384], F32, tag="at", name="atsb")
                    aTt = wp.tile([128, 384], F32, tag="aTt", name="aTt")
                    for ph in range(3):
                        pv = pvpool.tile([128, 1024], F32, tag="pv", name="pvps")
                        rc = wp.tile([128, 2], F32, tag="rc", name="rcsb")
                        for par in range(2):
                            h = 2 * ph + par
                            for c in range(3):
                                nc.tensor.matmul(
                                    pv[:, par * 512:par * 512 + 65],
                                    lhsT=e_tiles[c][:, (2 * ph) * 128 + par * 128:
                                                    (2 * ph) * 128 + (par + 1) * 128],
                                    rhs=v_sb[:, (bw * 3 + c) * 390 + h * 65:
                                             (bw * 3 + c) * 390 + (h + 1) * 65],
                                    start=(c == 0), stop=(c == 2),
                                )
                            nc.vector.reciprocal_approx_fast(
                                out=rc[:, par:par + 1],
                                in_=pv[:, par * 512 + 64:par * 512 + 65])
                        pv3 = pv[:].rearrange("p (b q) -> p b q", q=512)[:, :, 0:64]
                        rc3 = rc[:].rearrange("p (h o) -> p h o", o=1)
                        at3 = at[:, ph * 128:(ph + 1) * 128].rearrange(
                            "p (h e) -> p h e", e=64)
                        a, b = broadcast_tensor_aps(pv3, rc3)
                        nc.vector.tensor_mul(out=at3, in0=a, in1=b)
                    # transpose attn [128 q, 384 f] -> attnT tiles [128 f, 128 q]
                    for d3 in range(3):
                        trp = trpool.tile([128, 128], F32, tag="tr", name="trps")
                        nc.tensor.transpose(
                            out=trp[:], in_=at[:, d3 * 128:(d3 + 1) * 128],
                            identity=id_sb[:])
                        nc.scalar.copy(
                            out=aTt[:, d3 * 128:(d3 + 1) * 128], in_=trp[:])
                    fps = fpool.tile([128, 384], F32, tag="f", name="fps")
                    for d3 in range(3):
                        nc.tensor.matmul(
                            fps[:],
                            lhsT=aTt[:, d3 * 128:(d3 + 1) * 128],
                            rhs=w2_sb[d3][:],
                            start=(d3 == 0), stop=(d3 == 2),
                        )
                    ob = wp2.tile([128, 384], F32, tag="ob", name="obsb")
                    nc.scalar.copy(out=ob[:], in_=fps[:])
                    nc.sync.dma_start(out=out[bw], in_=ob[:])
    return nc


def full_neighbourhood_mask():
    """[4096, 4096] bool, True where key inside query's 7x7 clipped window."""
    hp = np.arange(64)
    sh = np.clip(hp - 3, 0, 57)
    hr = np.arange(64)
    rowv = (hr[None, :] >= sh[:, None]) & (hr[None, :] < (sh + 7)[:, None])
    m = rowv[:, None, :, None] & rowv[None, :, None, :]  # [qh, qw, kh, kw]
    return m.reshape(64 * 64, 64 * 64)


def core_mask_arr(g, fullmask):
    """bf16 [128, 8*3*128]: keys-on-partitions masks for row-group g.

    key index: window pos p = 128*c + ki, p = wl*24 + r (wl = key col
    - (8*bw - 4), r = local row); query index qi = qc*16 + qr.
    """
    out = np.zeros((8, 3, 128, 128), np.float32)
    qr = np.arange(16)
    qc = np.arange(8)
    for bw in range(8):
        p = np.arange(384)
        wl, r = p // 24, p % 24
        krow = 16 * g - 3 + r
        kcol = 8 * bw - 4 + wl
        kvalid = (krow >= 0) & (krow < 64) & (kcol >= 0) & (kcol < 64)
        ktok = np.clip(krow, 0, 63) * 64 + np.clip(kcol, 0, 63)
        qrow = 16 * g + qr
        qcol = 8 * bw + qc
        # qi = qc*16 + qr -> qc outer, qr inner
        qtok = (qrow[None, :] * 64 + qcol[:, None]).ravel()
        m = fullmask[qtok[None, :], ktok[:, None].astype(np.intp)]  # [384, 128]
        m = m & kvalid[:, None]
        out[bw] = m.reshape(3, 128, 128)
    return np.ascontiguousarray(
        out.transpose(2, 0, 1, 3).reshape(128, 8 * 3 * 128)
    ).astype(ml_dtypes.bfloat16)


_NC_CACHE = {}


def build():
    if "nc" not in _NC_CACHE:
        nc = bacc.Bacc("TRN2", target_bir_lowering=False, debug=False)
        emit(nc)
        nc.compile()
        _NC_CACHE["nc"] = nc
    return _NC_CACHE["nc"]


def make_in_maps(x, w_qkv, w_out):
    x = np.asarray(x, np.float32)
    wqkvT = np.ascontiguousarray(np.asarray(w_qkv, np.float32).T)
    woutT = np.ascontiguousarray(np.asarray(w_out, np.float32).T)
    ident = np.eye(128, dtype=np.float32)
    fullmask = full_neighbourhood_mask()
    gmasks = [core_mask_arr(g, fullmask) for g in range(4)]
    in_maps = []
    for core in range(NCORES):
        b, g = core // 4, core % 4
        rows = np.arange(16 * g - 3, 16 * g + 21)
        xs = np.zeros((24, 72, D), np.float32)  # [r, wc, D]
        valid = (rows >= 0) & (rows < 64)
        xs[valid, 4:68] = x[b, rows[valid]]
        # col-major tokens: token = wc*24 + r
        xT = np.ascontiguousarray(xs.transpose(2, 1, 0).reshape(D, 72 * 24))
        in_maps.append({
            "xT": xT, "wqkvT": wqkvT, "woutT": woutT,
            "ident": ident, "mask": gmasks[g],
        })
    return in_maps


def gather(results):
    full = np.zeros((2, 64, 64, D), np.float32)
    for core in range(NCORES):
        b, g = core // 4, core % 4
        o = results[core]["out"]  # [bw, qi = qc*16 + qr, f]
        o = o.reshape(8, 8, 16, D).transpose(2, 0, 1, 3).reshape(16, 64, D)
        full[b, 16 * g:16 * g + 16] = o
    return full


def kernel(x, w_qkv, w_out):
    nc = build()
    in_maps = make_in_maps(x, w_qkv, w_out)
    res = run_bass_kernel_spmd(nc, in_maps, core_ids=list(range(NCORES)))
    return gather(res.results)


def np_reference(x, w_qkv, w_out):
    """Plain-numpy port of reference.py for offline validation."""
    B, H, W, Dd = x.shape
    nh = Dd // E
    N = H * W
    qkv = x.reshape(B * N, Dd) @ w_qkv.T
    qkv = qkv.reshape(B, N, 3, nh, E).transpose(2, 0, 3, 1, 4)
    q, k, v = qkv[0], qkv[1], qkv[2]
    m = full_neighbourhood_mask()
    s = np.einsum("bnqe,bnke->bnqk", q, k) * (1.0 / np.sqrt(E))
    s = np.where(m[None, None], s, -np.inf)
    s = s - s.max(-1, keepdims=True)
    p = np.exp(s)
    p /= p.sum(-1, keepdims=True)
    o = np.einsum("bnqk,bnke->bnqe", p, v)
    o = o.transpose(0, 2, 1, 3).reshape(B, H, W, Dd)
    return o @ w_out.T


if __name__ == "__main__":
    from concourse.bass_interp import CoreSim
    rng = np.random.default_rng(0)
    x = rng.standard_normal((2, 64, 64, D), dtype=np.float32)
    w_qkv = (rng.standard_normal((3 * D, D)) * 0.02).astype(np.float32)
    w_out = (rng.standard_normal((D, D)) * 0.02).astype(np.float32)
    expected = np_reference(x, w_qkv, w_out)
    nc = build()
    in_maps = make_in_maps(x, w_qkv, w_out)
    core = int(sys.argv[1]) if len(sys.argv) > 1 else 0
    sim = CoreSim(nc)
    for kk, v in in_maps[core].items():
        sim.tensor(kk)[:] = v
    sim.simulate()
    got = np.array(sim.tensor("out"))
    b, g = core // 4, core % 4
    got = got.reshape(8, 8, 16, D).transpose(2, 0, 1, 3).reshape(16, 64, D)
    exp = expected[b, 16 * g:16 * g + 16]
    rel = np.linalg.norm(got - exp) / np.linalg.norm(exp)
    print(f"core {core}: rel_l2={rel:.3e} "
          f"absmax_rel={np.abs(got - exp).max() / np.abs(exp).max():.3e}")



# revision 22
# speedup vs baseline: 3.5220x; 3.5220x over previous
"""Neighbourhood attention block (7x7 window) on 8 Trainium2 NeuronCores.

Full inputs -> full output. Sharding: core = b*4 + g owns batch b and query
rows 16g..16g+15 (all 6 heads). Each core gets a 24-row halo slice of x,
transposed to feature-major and laid out in column-major token order
(token = wc*24 + r, wc = padded column 0..71, r = local row 0..23) so that
every 16-col x 24-row key window is a contiguous 384-token run: key chunks
(128 keys) and query blocks (128 queries = 8 cols x 16 rows) are then plain
2D slices, as required for matmul stationary operands.

All matmul operands are bf16 (x and the weights are pre-cast on the host),
so every matmul streams at 1 PE cycle per output row instead of fp32's 4.
Softmax runs without max-subtraction (scores are O(1)): keys on partitions,
probs = exp(s/8) * mask01 (bf16, one exp per head-pair covering all three
key chunks; the mask multiply runs on the otherwise idle Pool engine); the
denominator comes free from a ones-column appended to V (PV output col 64
of each 65-col head slot); reciprocal + normalization are per-partition ops
on the token-major PV output; attn is then transposed to feature-major by
the DMA XBAR (dma_start_transpose) instead of the PE for the out-projection.
"""
import sys

sys.path.insert(0, "/opt/trn_rl_repo")

import numpy as np
import ml_dtypes

import concourse.bass as bass
import concourse.mybir as mybir
from concourse import bacc
from concourse.tile import TileContext
from concourse.bass_utils import run_bass_kernel_spmd
from concourse.bass import broadcast_tensor_aps

F32 = mybir.dt.float32
BF16 = mybir.dt.bfloat16
AF = mybir.ActivationFunctionType

D = 384
NH = 6
E = 64
NCORES = 8
TOK = 1728          # 72 padded cols x 24 rows, column-major
SCALE = 0.125       # 1/sqrt(64)


def emit(nc):
    xT = nc.dram_tensor("xT", [D, TOK], BF16, kind="ExternalInput").ap()
    wqkvT = nc.dram_tensor("wqkvT", [D, 3 * D], BF16, kind="ExternalInput").ap()
    woutT = nc.dram_tensor("woutT", [D, D], BF16, kind="ExternalInput").ap()
    mask = nc.dram_tensor("mask", [128, 8 * 3 * 128], BF16, kind="ExternalInput").ap()
    ident = nc.dram_tensor("ident", [128, 128], BF16, kind="ExternalInput").ap()
    out = nc.dram_tensor("out", [8, 128, D], F32, kind="ExternalOutput").ap()

    with TileContext(nc) as tc:
        with tc.tile_pool(name="persist", bufs=1) as pp:
            xT_sb = [pp.tile([128, TOK], BF16, tag=f"xT{i}", name=f"xT{i}")
                     for i in range(3)]
            w1_sb = [pp.tile([128, 3 * D], BF16, tag=f"w1{i}", name=f"w1{i}")
                     for i in range(3)]
            w2_sb = [pp.tile([128, D], BF16, tag=f"w2{i}", name=f"w2{i}")
                     for i in range(3)]
            mk_sb = pp.tile([128, 8 * 3 * 128], BF16, tag="mk", name="mksb")
            qT_sb = pp.tile([128, 3 * 1024], BF16, tag="qT", name="qTsb")
            kT_sb = [pp.tile([128, TOK], BF16, tag=f"kT{i}", name=f"kT{i}")
                     for i in range(3)]
            # v: 14 distinct 128-token chunks; the 12 odd-window slots are
            # partition-shifted SBUF->SBUF DMA copies of those
            v_sb = pp.tile([128, 14 * 390], BF16, tag="v", name="vsb")
            vo_sb = pp.tile([128, 12 * 390], BF16, tag="vo", name="vosb")
            id_sb = pp.tile([128, 128], BF16, tag="id", name="idsb")

            # split input loads across the two DMA-trigger engines (SP and
            # Act) — each engine runs its DMAs serially in this cost model
            for i in range(3):
                nc.sync.dma_start(out=xT_sb[i][:], in_=xT[i * 128:(i + 1) * 128, :])
                nc.scalar.dma_start(out=w1_sb[i][:],
                                    in_=wqkvT[i * 128:(i + 1) * 128, :])
            nc.sync.dma_start(out=mk_sb[:], in_=mask[:])
            for i in range(3):
                nc.sync.dma_start(out=w2_sb[i][:], in_=woutT[i * 128:(i + 1) * 128, :])
            nc.sync.dma_start(out=id_sb[:], in_=ident[:])

            # ones-columns of v (col 64 of each 65-col head slot)
            vv = v_sb[:].rearrange("p (c h e) -> p c h e", h=NH, e=65)
            nc.gpsimd.memset(vv[:, :, :, 64:65], 1.0)
            # k at the 4 pad columns each side is never computed; zero it so
            # exp() of those (masked) scores stays finite
            for f in range(3):
                nc.gpsimd.memset(kT_sb[f][:, 0:96], 0.0)
                nc.gpsimd.memset(kT_sb[f][:, 1632:1728], 0.0)

            with tc.tile_pool(name="ppool", bufs=2, space="PSUM") as ppool, \
                 tc.tile_pool(name="spool", bufs=2, space="PSUM") as spool, \
                 tc.tile_pool(name="pvpool", bufs=1, space="PSUM") as pvpool, \
                 tc.tile_pool(name="epool", bufs=12) as epool, \
                 tc.tile_pool(name="wpool", bufs=3) as wpool, \
                 tc.tile_pool(name="aTpool", bufs=3) as aTpool, \
                 tc.tile_pool(name="wpool2", bufs=2) as wpool2:
                # ---- stage emitters for the software-pipelined schedule ----
                e_live = {}
                aT_live = {}

                def emit_qT():
                    # owned tokens (cols 4..67, rows 3..18), col-major
                    for ph in range(3):
                        for t in range(2):
                            ps = ppool.tile([128, 512], F32, tag="pp", name="pps")
                            for d in range(3):
                                xv = xT_sb[d][:].rearrange("p (w r) -> p w r",
                                                           r=24)
                                nc.tensor.matmul(
                                    ps[:],
                                    lhsT=w1_sb[d][:, ph * 128:(ph + 1) * 128],
                                    rhs=xv[:, 4 + 32 * t:4 + 32 * (t + 1), 3:19],
                                    start=(d == 0), stop=(d == 2),
                                )
                            nc.vector.tensor_copy(
                                out=qT_sb[:, ph * 1024 + t * 512:
                                          ph * 1024 + (t + 1) * 512],
                                in_=ps[:])

                def emit_kT(t):
                    # k^T tile t over the 1536 real-column tokens
                    for f in range(3):
                        ps = ppool.tile([128, 512], F32, tag="pp", name="pps")
                        for d in range(3):
                            nc.tensor.matmul(
                                ps[:],
                                lhsT=w1_sb[d][:, 384 + f * 128:384 + (f + 1) * 128],
                                rhs=xT_sb[d][:, 96 + t * 512:96 + (t + 1) * 512],
                                start=(d == 0), stop=(d == 2),
                            )
                        nc.scalar.copy(
                            out=kT_sb[f][:, 96 + t * 512:96 + (t + 1) * 512],
                            in_=ps[:])

                def emit_vd(j):
                    # distinct v chunk j: tokens [128j, 128(j+1)); chunk 13
                    # only has 64 real tokens
                    np_ = 64 if j == 13 else 128
                    ps = ppool.tile([128, 512], F32, tag="pp", name="pps")
                    for d in range(3):
                        nc.tensor.matmul(
                            ps[0:np_, 0:384],
                            lhsT=xT_sb[d][:, 128 * j:128 * j + np_],
                            rhs=w1_sb[d][:, 768:1152],
                            start=(d == 0), stop=(d == 2),
                        )
                    nc.vector.tensor_copy(
                        out=vv[0:np_, j, :, 0:64],
                        in_=ps[0:np_, 0:384].rearrange("p (h e) -> p h e", e=64))

                def emit_vodd(bw):
                    # odd bw: window chunks straddle the 128 grid by 64
                    # tokens; build the 3 slots by partition-shifted copies
                    for c in range(3):
                        j = (192 * bw + 128 * c - 64) // 128
                        s = 3 * ((bw - 1) // 2) + c
                        dst = vo_sb[:, s * 390:(s + 1) * 390]
                        nc.sync.dma_start(
                            out=dst[0:64, :],
                            in_=v_sb[64:128, j * 390:(j + 1) * 390])
                        nc.sync.dma_start(
                            out=dst[64:128, :],
                            in_=v_sb[0:64, (j + 1) * 390:(j + 2) * 390])

                def vslot(bw, c):
                    # key-chunk slot [128 keys, 390] for window chunk (bw, c)
                    if bw % 2 == 0:
                        a = 3 * bw // 2 + c
                        return v_sb[:, a * 390:(a + 1) * 390]
                    s = 3 * ((bw - 1) // 2) + c
                    return vo_sb[:, s * 390:(s + 1) * 390]

                def emit_scores(bw):
                    # one tile per head-pair over all 3 key chunks
                    # (head par -> bank par, chunk c -> col slot c)
                    k0 = 192 * bw
                    e_tiles = []
                    for ph in range(3):
                        sps = spool.tile([128, 1024], F32, tag="s", name="sps")
                        for c in range(3):
                            for par in range(2):
                                nc.tensor.matmul(
                                    sps[:, par * 512 + c * 128:
                                        par * 512 + (c + 1) * 128],
                                    lhsT=kT_sb[ph][par * 64:par * 64 + 64,
                                                   k0 + 128 * c:k0 + 128 * (c + 1)],
                                    rhs=qT_sb[par * 64:par * 64 + 64,
                                              ph * 1024 + bw * 128:
                                              ph * 1024 + (bw + 1) * 128],
                                    start=True, stop=True,
                                    tile_position=(par * 64, 0),
                                )
                        e_sb = epool.tile([128, 768], BF16, tag="e", name="esb")
                        s4 = sps[:].rearrange("p (b c q) -> p b c q",
                                              b=2, c=4)[:, :, 0:3, :]
                        e4 = e_sb[:].rearrange("p (b c q) -> p b c q", b=2, c=3)
                        nc.scalar.activation(out=e4, in_=s4, func=AF.Exp,
                                             scale=SCALE)
                        m4 = mk_sb[:, bw * 384:(bw + 1) * 384].rearrange(
                            "p (o c q) -> p o c q", o=1, c=3)
                        a, b = broadcast_tensor_aps(e4, m4)
                        nc.gpsimd.tensor_mul(out=e4, in0=a, in1=b)
                        e_tiles.append(e_sb)
                    e_live[bw] = e_tiles

                def emit_pv(bw, pe_transpose=False):
                    # PV: head h -> bank h//3, slot h%3 (65 cols, col 64 = Z)
                    e_tiles = e_live.pop(bw)
                    pv = pvpool.tile([128, 1024], F32, tag="pv", name="pvps")
                    for h in range(NH):
                        ph, par = h // 2, h % 2
                        o = (h // 3) * 512 + (h % 3) * 65
                        for c in range(3):
                            nc.tensor.matmul(
                                pv[:, o:o + 65],
                                lhsT=e_tiles[ph][:, par * 384 + c * 128:
                                                 par * 384 + (c + 1) * 128],
                                rhs=vslot(bw, c)[:, h * 65:(h + 1) * 65],
                                start=(c == 0), stop=(c == 2),
                            )
                    rc = wpool.tile([128, 8], F32, tag="rc", name="rcsb")
                    pvz = pv[:].rearrange("p (b s) -> p b s", b=2)
                    nc.vector.reciprocal_approx_fast(
                        out=rc[:, 0:6].rearrange("p (b h) -> p b h", b=2),
                        in_=pvz[:, :, 64:195:65])
                    at = wpool.tile([128, 384], BF16, tag="at", name="atsb")
                    pv4 = pvz[:, :, 0:195] \
                        .rearrange("p b (h e) -> p b h e", e=65)[:, :, :, 0:64]
                    rc4 = rc[:, 0:6].rearrange("p (b h o) -> p b h o", b=2, o=1)
                    at4 = at[:].rearrange("p (b h e) -> p b h e", b=2, e=64)
                    aa, bb = broadcast_tensor_aps(pv4, rc4)
                    nc.vector.tensor_mul(out=at4, in0=aa, in1=bb)
                    # transpose attn [128 q, 384 f] -> [f, q] via the DMA XBAR
                    # (PE + identity for the last block: no DMA init latency
                    # left to hide in the drain)
                    aT = aTpool.tile([128, 384], BF16, tag="aT", name="aTsb")
                    if pe_transpose:
                        trp = spool.tile([128, 2048], BF16, tag="s", name="sps")
                        for d3 in range(3):
                            nc.tensor.transpose(
                                out=trp[:, d3 * 128:(d3 + 1) * 128],
                                in_=at[:, d3 * 128:(d3 + 1) * 128],
                                identity=id_sb[:])
                            nc.scalar.copy(
                                out=aT[:, d3 * 128:(d3 + 1) * 128],
                                in_=trp[:, d3 * 128:(d3 + 1) * 128])
                    else:
                        for d3 in range(3):
                            nc.sync.dma_start_transpose(
                                out=aT[:, d3 * 128:(d3 + 1) * 128],
                                in_=at[:, d3 * 128:(d3 + 1) * 128])
                    aT_live[bw] = aT

                def emit_op(bw):
                    aT = aT_live.pop(bw)
                    fps = ppool.tile([128, 512], F32, tag="pp", name="pps")
                    for d3 in range(3):
                        nc.tensor.matmul(
                            fps[:, 0:384],
                            lhsT=aT[:, d3 * 128:(d3 + 1) * 128],
                            rhs=w2_sb[d3][:],
                            start=(d3 == 0), stop=(d3 == 2),
                        )
                    ob = wpool2.tile([128, 384], F32, tag="ob", name="obsb")
                    nc.vector.tensor_copy(out=ob[:], in_=fps[:, 0:384])
                    nc.sync.dma_start(out=out[bw], in_=ob[:])

                # scores(bw) needs kT tile t up to (192*bw+384-97)//512;
                # exp/mask trail scores, PV trails by >=2 lines, the out
                # projection trails PV by >=2 lines (hides the ~1.8us DMA
                # transpose latency), v(bw) runs >=1 line ahead of pv(bw)
                emit_qT()
                emit_kT(0); emit_vd(0); emit_vd(1)
                emit_scores(0); emit_vd(2); emit_vd(3)
                emit_kT(1); emit_vd(4); emit_vd(5); emit_vodd(1)
                emit_scores(1); emit_vd(6); emit_vd(7)
                emit_scores(2); emit_vd(8); emit_vodd(3)
                emit_kT(2); emit_scores(3); emit_vd(9); emit_pv(0)
                emit_scores(4); emit_vd(10); emit_vd(11); emit_pv(1)
                emit_vodd(5)
                emit_scores(5); emit_vd(12); emit_vd(13); emit_pv(2)
                emit_vodd(7); emit_op(0)
                emit_scores(6); emit_pv(3); emit_op(1)
                emit_scores(7); emit_pv(4); emit_op(2)
                emit_pv(5); emit_op(3)
                emit_pv(6); emit_op(4)
                emit_pv(7, pe_transpose=True); emit_op(5)
                emit_op(6)
                emit_op(7)
    return nc


def full_neighbourhood_mask():
    """[4096, 4096] bool, True where key inside query's 7x7 clipped window."""
    hp = np.arange(64)
    sh = np.clip(hp - 3, 0, 57)
    hr = np.arange(64)
    rowv = (hr[None, :] >= sh[:, None]) & (hr[None, :] < (sh + 7)[:, None])
    m = rowv[:, None, :, None] & rowv[None, :, None, :]  # [qh, qw, kh, kw]
    return m.reshape(64 * 64, 64 * 64)


def core_mask_arr(g, fullmask):
    """bf16 [128, 8*3*128]: keys-on-partitions masks for row-group g.

    key index: window pos p = 128*c + ki, p = wl*24 + r (wl = key col
    - (8*bw - 4), r = local row); query index qi = qc*16 + qr.
    """
    out = np.zeros((8, 3, 128, 128), np.float32)
    qr = np.arange(16)
    qc = np.arange(8)
    for bw in range(8):
        p = np.arange(384)
        wl, r = p // 24, p % 24
        krow = 16 * g - 3 + r
        kcol = 8 * bw - 4 + wl
        kvalid = (krow >= 0) & (krow < 64) & (kcol >= 0) & (kcol < 64)
        ktok = np.clip(krow, 0, 63) * 64 + np.clip(kcol, 0, 63)
        qrow = 16 * g + qr
        qcol = 8 * bw + qc
        # qi = qc*16 + qr -> qc outer, qr inner
        qtok = (qrow[None, :] * 64 + qcol[:, None]).ravel()
        m = fullmask[qtok[None, :], ktok[:, None].astype(np.intp)]  # [384, 128]
        m = m & kvalid[:, None]
        out[bw] = m.reshape(3, 128, 128)
    return np.ascontiguousarray(
        out.transpose(2, 0, 1, 3).reshape(128, 8 * 3 * 128)
    ).astype(ml_dtypes.bfloat16)


_NC_CACHE = {}


def build():
    if "nc" not in _NC_CACHE:
        nc = bacc.Bacc("TRN2", target_bir_lowering=False, debug=False)
        emit(nc)
        nc.compile()
        _NC_CACHE["nc"] = nc
    return _NC_CACHE["nc"]


def make_in_maps(x, w_qkv, w_out):
    x = np.asarray(x, np.float32)
    wqkvT = np.ascontiguousarray(np.asarray(w_qkv, np.float32).T).astype(
        ml_dtypes.bfloat16)
    woutT = np.ascontiguousarray(np.asarray(w_out, np.float32).T).astype(
        ml_dtypes.bfloat16)
    ident = np.eye(128, dtype=ml_dtypes.bfloat16)
    fullmask = full_neighbourhood_mask()
    gmasks = [core_mask_arr(g, fullmask) for g in range(4)]
    in_maps = []
    for core in range(NCORES):
        b, g = core // 4, core % 4
        rows = np.arange(16 * g - 3, 16 * g + 21)
        xs = np.zeros((24, 72, D), np.float32)  # [r, wc, D]
        valid = (rows >= 0) & (rows < 64)
        xs[valid, 4:68] = x[b, rows[valid]]
        # col-major tokens: token = wc*24 + r
        xT = np.ascontiguousarray(
            xs.transpose(2, 1, 0).reshape(D, 72 * 24)).astype(ml_dtypes.bfloat16)
        in_maps.append({
            "xT": xT, "wqkvT": wqkvT, "woutT": woutT, "mask": gmasks[g],
            "ident": ident,
        })
    return in_maps


def gather(results):
    full = np.zeros((2, 64, 64, D), np.float32)
    for core in range(NCORES):
        b, g = core // 4, core % 4
        o = results[core]["out"]  # [bw, qi = qc*16 + qr, f]
        o = o.reshape(8, 8, 16, D).transpose(2, 0, 1, 3).reshape(16, 64, D)
        full[b, 16 * g:16 * g + 16] = o
    return full


def kernel(x, w_qkv, w_out):
    nc = build()
    in_maps = make_in_maps(x, w_qkv, w_out)
    res = run_bass_kernel_spmd(nc, in_maps, core_ids=list(range(NCORES)))
    return gather(res.results)


def np_reference(x, w_qkv, w_out):
    """Plain-numpy port of reference.py for offline validation."""
    B, H, W, Dd = x.shape
    nh = Dd // E
    N = H * W
    qkv = x.reshape(B * N, Dd) @ w_qkv.T
    qkv = qkv.reshape(B, N, 3, nh, E).transpose(2, 0, 3, 1, 4)
    q, k, v = qkv[0], qkv[1], qkv[2]
    m = full_neighbourhood_mask()
    s = np.einsum("bnqe,bnke->bnqk", q, k) * (1.0 / np.sqrt(E))
    s = np.where(m[None, None], s, -np.inf)
    s = s - s.max(-1, keepdims=True)
    p = np.exp(s)
    p /= p.sum(-1, keepdims=True)
    o = np.einsum("bnqk,bnke->bnqe", p, v)
    o = o.transpose(0, 2, 1, 3).reshape(B, H, W, Dd)
    return o @ w_out.T


if __name__ == "__main__":
    from concourse.bass_interp import CoreSim
    rng = np.random.default_rng(0)
    x = rng.standard_normal((2, 64, 64, D), dtype=np.float32)
    w_qkv = (rng.standard_normal((1152, 384)) * 0.02).astype(np.float32)
    w_out = (rng.standard_normal((384, 384)) * 0.02).astype(np.float32)
    expected = np_reference(x, w_qkv, w_out)
    nc = build()
    in_maps = make_in_maps(x, w_qkv, w_out)
    core = int(sys.argv[1]) if len(sys.argv) > 1 else 0
    sim = CoreSim(nc)
    for kk, v in in_maps[core].items():
        sim.tensor(kk)[:] = v
    sim.simulate()
    got = np.array(sim.tensor("out"))
    b, g = core // 4, core % 4
    got = got.reshape(8, 8, 16, D).transpose(2, 0, 1, 3).reshape(16, 64, D)
    exp = expected[b, 16 * g:16 * g + 16]
    rel = np.linalg.norm(got - exp) / np.linalg.norm(exp)
    print(f"core {core}: rel_l2={rel:.3e} "
          f"absmax_rel={np.abs(got - exp).max() / np.abs(exp).max():.3e} "
          f"time={sim.time}")
